# revision 13
# baseline (speedup 1.0000x reference)
"""GAT layer kernel for Trainium2, distributed over 8 NeuronCores.

Reference computation (per graph-attention layer):
    h = x @ W                                   [n, d]
    e = (h@a1)[:,None] + (h@a2)[None,:] + b     [n, n]
    e = leaky_relu(e, 0.2)
    e = where(adj == 0, -inf, e)
    alpha = softmax(e, axis=1)
    alpha *= exp(-dist) * (clip(cos(angle), 0) + 1e-6)
    alpha /= sum(alpha, axis=1)
    out = alpha @ h                             [n, d]

Distribution: each core owns a 512-row block of the [n, n] attention
matrix.  The softmax normalizer cancels against the final renorm (both
divide the same row), so the kernel computes unnormalized
w = exp(leaky(e) - D) * (cos(angle) + 1e-6) with D = dist + 1e4*(1-adj)
(additive adjacency mask; exp underflows to exactly 0 on masked entries)
and a single division by the row sum at the end.

On-chip layout puts j (columns) on partitions and i (rows) on the free
dim, so the final contraction w.T-block @ [h | 1] runs natively on the
tensor engine and row sums fall out of the ones column.  Host-side work
is limited to layout marshaling: transposed row-blocks of the two
streamed matrices, x.T, and the tiny reassociated vectors W@a1 / W@a2.
"""

import numpy as np

import concourse.bass as bass
import concourse.bacc as bacc
import concourse.mybir as mybir
import concourse.tile as tile

N = 4096
DIM = 128
NCORES = 8
R = N // NCORES          # rows per core (512)
PJ = 128                 # j per partition tile
NJT = N // PJ            # 32 j-tiles
NEG_SLOPE = 0.2
MASK = 1.0e4
F32 = mybir.dt.float32
AF = mybir.ActivationFunctionType
ALU = mybir.AluOpType
PSUM = bass.MemorySpace.PSUM


def build_nc(n=N, dim=DIM, r=R, grp=2, lrelu_mode="act_relu", chop=4):
    """Build the per-core Bass program (identical on every core).

    lrelu_mode:
      "act_relu": r2 = Relu(0.8*e) on ACT; leaky = 0.2*e + r2 on DVE
      "dve_max":  leaky = max(0.2*e, e) in one DVE op (dual-PSUM read)
      "act_lrelu": single ACT Lrelu op (HW semantics must be probed)
    chop: split DVE elementwise ops into this many free-dim chunks
          (dodges the per-op pipeline DRAIN which scales with op size)
    """
    njt = n // PJ
    ngrp = njt // grp
    fr = grp * r                 # free elems per group op
    nib = r // PJ                # i sub-blocks per core (4)

    nc = bacc.Bacc("TRN2", target_bir_lowering=False, debug=False)

    xT = nc.dram_tensor("xT", [dim, n], F32, kind="ExternalInput")
    xTb = nc.dram_tensor("xTb", [dim, r], F32, kind="ExternalInput")
    W = nc.dram_tensor("W", [dim, dim], F32, kind="ExternalInput")
    w1 = nc.dram_tensor("w1", [dim, 1], F32, kind="ExternalInput")
    w2 = nc.dram_tensor("w2", [dim, 1], F32, kind="ExternalInput")
    bb = nc.dram_tensor("bb", [1, 1], F32, kind="ExternalInput")
    DT = nc.dram_tensor("DT", [n, r], F32, kind="ExternalInput")
    AT = nc.dram_tensor("AT", [n, r], F32, kind="ExternalInput")
    ones1 = nc.dram_tensor("ones1", [1, n], F32, kind="ExternalInput")
    out = nc.dram_tensor("out", [r, dim], F32, kind="ExternalOutput")
    sdram = nc.dram_tensor("sdram", [1, r], F32)

    DTg = DT[:].rearrange("(G a p) i -> G p a i", a=grp, p=PJ)
    ATg = AT[:].rearrange("(G a p) i -> G p a i", a=grp, p=PJ)

    with tile.TileContext(nc) as tc:
        # ---------- long-lived tensors ----------
        cpool = tc.alloc_tile_pool(name="const", bufs=1)
        t2_sb = cpool.tile([2, n], F32, tag="t2")     # row0 t+b, row1 ones
        s2_sb = cpool.tile([2, r], F32, tag="s2")     # row0 ones, row1 s
        h_sb = cpool.tile([PJ, njt, dim + 1], F32, tag="h")  # [h | 1]
        pio2_sb = cpool.tile([PJ, 1], F32, tag="pio2")

        nc.vector.memset(pio2_sb[:], float(np.pi / 2))
        # engine writes must start at partition 0, so row 1 of t2/s2 is
        # filled via DMA (ones input / DRAM bounce of the s row)
        nc.sync.dma_start(t2_sb[1:2, :], ones1[:])
        nc.vector.memset(s2_sb[0:1, :], 1.0)
        nc.vector.memset(h_sb[:, :, dim:dim + 1], 1.0)

        # ---------- prologue: t = x@w2 + b, s = x@w1, h = x@W ----------
        plpool = tc.alloc_tile_pool(name="prolsb", bufs=1)
        ppool = tc.alloc_tile_pool(name="prolps", bufs=2, space=PSUM)

        xT_sb = plpool.tile([dim, n], F32, tag="xT")
        nc.sync.dma_start(xT_sb[:], xT[:])
        xTb_sb = plpool.tile([dim, r], F32, tag="xTb")
        nc.sync.dma_start(xTb_sb[:], xTb[:])
        W_sb = plpool.tile([dim, dim], F32, tag="W")
        nc.sync.dma_start(W_sb[:], W[:])
        w1_sb = plpool.tile([dim, 1], F32, tag="w1")
        nc.sync.dma_start(w1_sb[:], w1[:])
        w2_sb = plpool.tile([dim, 1], F32, tag="w2")
        nc.sync.dma_start(w2_sb[:], w2[:])
        b_sb = plpool.tile([1, 1], F32, tag="b")
        nc.sync.dma_start(b_sb[:], bb[:])

        tch = min(512, n)
        for k in range(n // tch):
            tp = ppool.tile([1, tch], F32, tag="tp", name=f"tp{k}")
            nc.tensor.matmul(tp[:], w2_sb[:], xT_sb[:, k * tch:(k + 1) * tch])
            nc.scalar.activation(t2_sb[0:1, k * tch:(k + 1) * tch], tp[:],
                                 AF.Identity, bias=b_sb[:])
        sch = min(512, r)
        srow = plpool.tile([1, r], F32, tag="srow")
        for k in range(r // sch):
            sp = ppool.tile([1, sch], F32, tag="sp", name=f"sp{k}")
            nc.tensor.matmul(sp[:], w1_sb[:], xTb_sb[:, k * sch:(k + 1) * sch])
            nc.scalar.activation(srow[0:1, k * sch:(k + 1) * sch], sp[:],
                                 AF.Identity)
        nc.sync.dma_start(sdram[:], srow[:])
        nc.sync.dma_start(s2_sb[1:2, :], sdram[:])

        for jt in range(njt):
            hp = ppool.tile([PJ, dim], F32, tag="hp", name=f"hp{jt}")
            nc.tensor.matmul(hp[:], xT_sb[:, jt * PJ:(jt + 1) * PJ], W_sb[:])
            nc.vector.tensor_copy(h_sb[:, jt, 0:dim], hp[:])

        ppool.release()
        plpool.release()

        # ---------- main-loop pools ----------
        dpool = tc.alloc_tile_pool(name="dstream", bufs=3)
        apool = tc.alloc_tile_pool(name="astream", bufs=3)
        wpool = tc.alloc_tile_pool(name="work", bufs=2)
        upool = tc.alloc_tile_pool(name="uhold", bufs=ngrp)
        cospool = tc.alloc_tile_pool(name="cos", bufs=4)
        opool = tc.alloc_tile_pool(name="epi", bufs=4)
        accpool = tc.alloc_tile_pool(name="acc", bufs=1, space=PSUM)
        epool = tc.alloc_tile_pool(name="eps", bufs=2, space=PSUM)

        acc = [accpool.tile([PJ, dim + 1], F32, tag=f"acc{ib}", name=f"acc{ib}")
               for ib in range(nib)]

        def chopped(op):
            cw = fr // chop
            for cc in range(chop):
                op(slice(cc * cw, (cc + 1) * cw))

        def emit_exp_side(g):
            dt = dpool.tile([PJ, grp, r], F32, tag="dt", name=f"dt{g}")
            nc.sync.dma_start(dt[:], DTg[g])
            dtf = dt[:].rearrange("p a i -> p (a i)")

            e_ps = epool.tile([PJ, grp, r], F32, tag="e", name=f"e{g}")
            for a in range(grp):
                jt = g * grp + a
                nc.tensor.matmul(e_ps[:, a, :],
                                 t2_sb[:, jt * PJ:(jt + 1) * PJ], s2_sb[:])
            epf = e_ps[:].rearrange("p a i -> p (a i)")

            gt = wpool.tile([PJ, fr], F32, tag="g", name=f"g{g}")
            if lrelu_mode == "act_lrelu":
                e2 = wpool.tile([PJ, fr], F32, tag="e2", name=f"e2{g}")
                nc.scalar.activation(e2[:], epf, AF.Lrelu, alpha=NEG_SLOPE)
                chopped(lambda s: nc.vector.scalar_tensor_tensor(
                    gt[:, s], e2[:, s], 1.0, dtf[:, s],
                    ALU.bypass, ALU.subtract))
            elif lrelu_mode == "dve_max":
                e2 = wpool.tile([PJ, fr], F32, tag="e2", name=f"e2{g}")
                chopped(lambda s: nc.vector.scalar_tensor_tensor(
                    e2[:, s], epf[:, s], NEG_SLOPE, epf[:, s],
                    ALU.mult, ALU.max))
                chopped(lambda s: nc.vector.scalar_tensor_tensor(
                    gt[:, s], e2[:, s], 1.0, dtf[:, s],
                    ALU.bypass, ALU.subtract))
            else:  # act_relu
                r2 = wpool.tile([PJ, fr], F32, tag="r2", name=f"r2{g}")
                # r2 = relu(0.8*e);  leaky(e) = 0.2*e + r2
                nc.scalar.activation(r2[:], epf, AF.Relu, scale=1.0 - NEG_SLOPE)
                m = wpool.tile([PJ, fr], F32, tag="m", name=f"m{g}")
                chopped(lambda s: nc.vector.scalar_tensor_tensor(
                    m[:, s], epf[:, s], NEG_SLOPE, r2[:, s],
                    ALU.mult, ALU.add))
                chopped(lambda s: nc.vector.scalar_tensor_tensor(
                    gt[:, s], m[:, s], 1.0, dtf[:, s],
                    ALU.bypass, ALU.subtract))

            ut = upool.tile([PJ, grp, r], F32, tag="u", name=f"u{g}")
            nc.scalar.activation(ut[:].rearrange("p a i -> p (a i)"), gt[:],
                                 AF.Exp)
            return ut

        def emit_trig_side(g, ut):
            at = apool.tile([PJ, grp, r], F32, tag="at", name=f"at{g}")
            nc.sync.dma_start(at[:], ATg[g])
            ct = cospool.tile([PJ, fr], F32, tag="ct", name=f"ct{g}")
            nc.scalar.activation(ct[:], at[:].rearrange("p a i -> p (a i)"),
                                 AF.Sin, bias=pio2_sb[:])
            wt = wpool.tile([PJ, grp, r], F32, tag="wt", name=f"wt{g}")
            wtf = wt[:].rearrange("p a i -> p (a i)")
            utf = ut[:].rearrange("p a i -> p (a i)")
            chopped(lambda s: nc.vector.scalar_tensor_tensor(
                wtf[:, s], ct[:, s], 1.0e-6, utf[:, s],
                ALU.add, ALU.mult))
            for a in range(grp):
                jt = g * grp + a
                for ib in range(nib):
                    nc.tensor.matmul(
                        acc[ib][:],
                        wt[:, a, ib * PJ:(ib + 1) * PJ],
                        h_sb[:, jt, :],
                        start=(jt == 0), stop=(jt == njt - 1))

        # Emit all exp-table-set ACT work before all trig-set work (one
        # activation-table swap).
        us = [emit_exp_side(g) for g in range(ngrp)]
        for g in range(ngrp):
            emit_trig_side(g, us[g])

        # ---------- epilogue: out = num / (rowsum + 1e-9) ----------
        for ib in range(nib):
            rs = opool.tile([PJ, 1], F32, tag="rs", name=f"rs{ib}")
            nc.vector.tensor_scalar_add(rs[:], acc[ib][:, dim:dim + 1], 1.0e-9)
            rec = opool.tile([PJ, 1], F32, tag="rec", name=f"rec{ib}")
            nc.vector.reciprocal(rec[:], rs[:])
            ot = opool.tile([PJ, dim], F32, tag="ot", name=f"ot{ib}")
            nc.vector.tensor_scalar_mul(ot[:], acc[ib][:, 0:dim], rec[:])
            nc.sync.dma_start(out[ib * PJ:(ib + 1) * PJ, :], ot[:])

        epool.release()
        accpool.release()
        opool.release()
        cospool.release()
        upool.release()
        wpool.release()
        apool.release()
        dpool.release()
        cpool.release()

    nc.compile()
    return nc


_NC_CACHE = {}


def _get_nc(**kw):
    key = tuple(sorted(kw.items()))
    if key not in _NC_CACHE:
        _NC_CACHE[key] = build_nc(**kw)
    return _NC_CACHE[key]


def host_prep(x, adj, dist_mat, angle_mat, W, attn_w, attn_b, n=N, dim=DIM,
              ncores=NCORES):
    """Shard + marshal inputs into the per-core layout."""
    x = np.ascontiguousarray(np.asarray(x, dtype=np.float32))
    adj = np.asarray(adj)
    dist_mat = np.asarray(dist_mat, dtype=np.float32)
    angle_mat = np.asarray(angle_mat, dtype=np.float32)
    W = np.ascontiguousarray(np.asarray(W, dtype=np.float32))
    attn_w = np.asarray(attn_w, dtype=np.float32)
    attn_b = np.asarray(attn_b, dtype=np.float32)

    r = n // ncores
    xT = np.ascontiguousarray(x.T)                      # [dim, n]
    w1 = np.ascontiguousarray((W @ attn_w[:dim]).reshape(dim, 1))
    w2 = np.ascontiguousarray((W @ attn_w[dim:]).reshape(dim, 1))
    bb = attn_b.reshape(1, 1).astype(np.float32)

    # Additive adjacency mask folded into the dist term: exp(e - D)
    # underflows to exactly 0 where adj == 0.
    D = dist_mat + np.float32(MASK) * (1.0 - adj.astype(np.float32))

    in_maps = []
    for c in range(ncores):
        sl = slice(c * r, (c + 1) * r)
        in_maps.append({
            "ones1": np.ones((1, n), dtype=np.float32),
            "xT": xT,
            "xTb": np.ascontiguousarray(xT[:, sl]),
            "W": W,
            "w1": w1,
            "w2": w2,
            "bb": bb,
            "DT": np.ascontiguousarray(D[sl].T),
            "AT": np.ascontiguousarray(angle_mat[sl].T),
        })
    return in_maps


def kernel(x, adj, dist_mat, angle_mat, W, attn_w, attn_b):
    from concourse.bass_utils import run_bass_kernel_spmd

    nc = _get_nc()
    in_maps = host_prep(x, adj, dist_mat, angle_mat, W, attn_w, attn_b)
    res = run_bass_kernel_spmd(nc, in_maps, core_ids=list(range(NCORES)))
    return np.concatenate([res.results[c]["out"] for c in range(NCORES)], axis=0)
